# revision 86
# baseline (speedup 1.0000x reference)
"""Trainium2 Bass kernel for nn_MemorizingTransformer (retrieval_knn).

Sharding: 8 cores = 2 batches x 4 head-pairs. Each core computes attention for
its batch and 2 heads plus its slice of the output projection; the host sums
the 4 partial outputs per batch (the "all-reduce after to_out").

Per-core algorithm (n=2048 tokens, dh=64, 2 heads, kret=32 memories):
  - q/k/v projected with weights stationary (fp16) -> qT / (k|v)T.
  - norms computed in the transposed layout: DVE squares + ones-matmuls give
    1/|q_h| (with exp(scale) folded) and 1/|k| as rows; a host-provided
    selector matmul broadcasts the rows back onto the head partition halves
    so qhatT / khatT (key rows duplicated per head) are built by one DVE
    multiply per chunk, with no per-tile transposes.
  - one softmax shift C_MEM serves BOTH branches (local weights land around
    exp(-40..-80), still representable in bf16; f32 PSUM accumulates fine).
  - local attention runs TRANSPOSED: per key-tile one matmul gives simT; the
    rel-pos bias (fp8, causal -240) is ADDED into the same PSUM by an
    identity-stationary matmul; exp on Act gives bf16 weights that are the
    PV matmul's stationary directly. The PV is emitted one key-tile late so
    the PE never waits on Act. An extra ones-column of v accumulates the
    local partition sum.
  - memory attention runs on the TENSOR engine with block-diagonal packing:
    4 queries' 4x32 keys fill the 128 contraction partitions, one matmul
    (free=4) per group computes their sims into an E tile; exp + a {0,1}
    mask (zeroing cross-query terms) gives weights; per-group PV matmuls
    against host-packed values [128, 65] (65th col of ones = partition sum)
    accumulate a transposed moT that a single PE transpose-accumulate folds
    straight onto the local PV accumulator. mem_mask is folded into the
    packed values on the host (zeroed rows+ones == -inf logits).
  - combine is just reciprocal + scale; the output projection transposes a,
    multiplies by the Wout slice, and streams out per 128-token tile.
  - the per-chunk qhT/khT build is interleaved with the attention chunks
    (sharing PSUM bank tags) so phase A exposes only the projections;
    DMA traffic is spread over the three DMA queues in need-time order.
"""

import numpy as np
import ml_dtypes
from contextlib import ExitStack

import concourse.bass as bass
import concourse.bacc as bacc
import concourse.mybir as mybir
import concourse.tile as tile
from concourse.masks import make_identity

F32 = mybir.dt.float32
F32R = mybir.dt.float32r
BF16 = mybir.dt.bfloat16
F16 = mybir.dt.float16
FP8 = mybir.dt.float8e4
AX = mybir.AxisListType
OP = mybir.AluOpType
ACTF = mybir.ActivationFunctionType

P = 128
DIM = 512
DH = 64
KRET = 32
HPC = 2            # heads per core
NCORES = 8
C_MEM = 60.0       # single softmax shift for BOTH branches: mem logits
                   # ~N(0,20^2) stay under f32 exp overflow after -C_MEM
                   # (max arg ~ +45); local logits <= 20.6 give weights
                   # >= exp(-81) which bf16 still represents (min ~1e-38)


def bcast_mid(ap_2d, count):
    """[P, d] AP -> [P, count, d] AP broadcasting a new middle dim (step 0)."""
    return bass.AP(tensor=ap_2d.tensor, offset=ap_2d.offset,
                   ap=[list(ap_2d.ap[0]), [0, count], list(ap_2d.ap[1])])


def build_nc(n=2048):
    """Build the per-core Bass program (same NEFF for all 8 cores)."""
    nt = n // P               # 128-token tiles
    nq = n // 512             # 512-query chunks
    ng = P // 4               # 4-query groups per tile
    nc = bacc.Bacc("TRN2", target_bir_lowering=False, debug=False)

    xt_d = nc.dram_tensor("xt", (DIM, n), F16, kind="ExternalInput").ap()
    wq_d = nc.dram_tensor("wq", (DIM, HPC * DH), F16, kind="ExternalInput").ap()
    wkv_d = nc.dram_tensor("wkv", (DIM, 2 * DH), F16, kind="ExternalInput").ap()
    wout_d = nc.dram_tensor("wout", (HPC * DH, DIM), F16, kind="ExternalInput").ap()
    # norm-pass constants (f32r so the PE runs them at full rate):
    # Lq[p, h] = 1/scale_h^2 on head h's partition half; onesk = k-row mask;
    # Lsel routes rsq row 0 -> partitions 0:64 and row 1 -> 64:128;
    # Lk picks rsq row 32 (the k norms); Ldup duplicates khT onto both halves
    Lq_d = nc.dram_tensor("Lq", (P, HPC), F32R, kind="ExternalInput").ap()
    onesk_d = nc.dram_tensor("onesk", (P, 1), F32, kind="ExternalInput").ap()
    Lsel_d = nc.dram_tensor("Lsel", (34, P), F32R, kind="ExternalInput").ap()
    Lk_d = nc.dram_tensor("Lk", (34, DH), F32R, kind="ExternalInput").ap()
    Ldup_d = nc.dram_tensor("Ldup", (DH, P), F16, kind="ExternalInput").ap()
    rsqz_d = nc.dram_tensor("rsqz", (34, 512), F32R, kind="ExternalInput").ap()
    # memkT[t, h*64+d, g*128+32j+k] = memk[h, 128t+4g+j, k, d]
    memkT_d = nc.dram_tensor("memkT", (nt, P, KRET * P), F16, kind="ExternalInput").ap()
    # memvp[t, 32j+k, h, g, d] = memv[h, 128t+4g+j, k, d]; [...,64] = mask
    memvp_d = nc.dram_tensor("memvp", (nt, P, HPC, KRET, DH + 1), BF16,
                             kind="ExternalInput").ap()
    # maskc[32j+k, j'] = 1 if j == j' else 0
    maskc_d = nc.dram_tensor("maskc", (P, 4), BF16, kind="ExternalInput").ap()
    # biasT[h, c, j, i'] = bias[h, 512c+i', j], or -240 if causal-masked
    # (512c+i' < j); fp8 e4m3 (additive logit error ~1e-3), added into the
    # sim PSUM by an identity-stationary matmul on the otherwise idle PE
    bias_d = nc.dram_tensor("biasT", (HPC, nq, n, 512), FP8,
                            kind="ExternalInput").ap()
    out_d = nc.dram_tensor("out", (n, DIM), F16, kind="ExternalOutput").ap()

    with tile.TileContext(nc) as tc, ExitStack() as ctx:
        persist = ctx.enter_context(tc.tile_pool(name="persist", bufs=1))

        # ---- constants -------------------------------------------------
        id_f = persist.tile([P, P], F32)
        make_identity(nc, id_f)
        id_h = persist.tile([P, P], F16)
        make_identity(nc, id_h)
        id_8 = persist.tile([P, P], FP8)
        nc.vector.tensor_copy(id_8, id_h)
        wout_sb = persist.tile([P, DIM], F16)
        nc.sync.dma_start(out=wout_sb, in_=wout_d)
        maskc_sb = persist.tile([P, 4], BF16)
        nc.sync.dma_start(out=maskc_sb, in_=maskc_d)
        Lq = persist.tile([P, HPC], F32R)
        nc.sync.dma_start(out=Lq, in_=Lq_d)
        onesk = persist.tile([P, 1], F32)
        nc.sync.dma_start(out=onesk, in_=onesk_d)
        Lsel = persist.tile([34, P], F32R)
        nc.sync.dma_start(out=Lsel, in_=Lsel_d)
        Lk = persist.tile([34, DH], F32R)
        nc.sync.dma_start(out=Lk, in_=Lk_d)
        Ldup = persist.tile([DH, P], F16)
        nc.sync.dma_start(out=Ldup, in_=Ldup_d)
        negm_sb = persist.tile([P, 1], F32)
        nc.vector.memset(negm_sb, -C_MEM)

        # ---- persistent activations -----------------------------------
        qt_sb = persist.tile([P, n], F32)      # raw qT
        kvt_sb = persist.tile([P, n], F32)     # raw kT|vT
        qhT_c = [persist.tile([P, 512], F16, name=f"qhT{i}") for i in range(nq)]
        kh2T_c = [persist.tile([P, 512], F16, name=f"kh2T{i}") for i in range(nq)]
        vb_t = [persist.tile([P, DH + 1], BF16, name=f"vb{i}") for i in range(nt)]
        a_t = [persist.tile([P, P], F16, name=f"a{i}") for i in range(nt)]

        # ================= projections =================================
        with ExitStack() as actx:
            pa = actx.enter_context(tc.tile_pool(name="pa", bufs=1))
            wq_sb = pa.tile([P, DIM // P, HPC * DH], F16)
            nc.sync.dma_start(out=wq_sb, in_=wq_d.rearrange("(c p) m -> p c m", p=P))
            wkv_sb = pa.tile([P, DIM // P, 2 * DH], F16)
            nc.sync.dma_start(out=wkv_sb, in_=wkv_d.rearrange("(c p) m -> p c m", p=P))
            xt_sb = pa.tile([P, DIM // P, n], F16)
            xt_r = xt_d.rearrange("(c p) n -> p c n", p=P)
            for cc in range(DIM // P):
                q_eng = (nc.sync, nc.sync, nc.gpsimd, nc.scalar)[cc]
                q_eng.dma_start(out=xt_sb[:, cc, :], in_=xt_r[:, cc, :])

            with ExitStack() as pctx:
                psA = pctx.enter_context(tc.tile_pool(name="psA", bufs=1, space="PSUM"))
                q_ps = [psA.tile([P, 512], F32, tag=f"q{t}", name=f"q_ps{t}")
                        for t in range(nq)]
                kv_ps = [psA.tile([P, 512], F32, tag=f"kv{t}", name=f"kv_ps{t}")
                         for t in range(nq)]
                # chunk-major so chunk 0 finishes first and the build pass
                # pipelines behind the remaining projections
                for t in range(nq):
                    for c in range(DIM // P):
                        last = c == DIM // P - 1
                        nc.tensor.matmul(q_ps[t], lhsT=wq_sb[:, c, :],
                                         rhs=xt_sb[:, c, bass.ts(t, 512)],
                                         start=(c == 0), stop=last)
                    for c in range(DIM // P):
                        last = c == DIM // P - 1
                        nc.tensor.matmul(kv_ps[t], lhsT=wkv_sb[:, c, :],
                                         rhs=xt_sb[:, c, bass.ts(t, 512)],
                                         start=(c == 0), stop=last)
                    nc.vector.tensor_copy(qt_sb[:, bass.ts(t, 512)], q_ps[t])
                    nc.vector.tensor_copy(kvt_sb[:, bass.ts(t, 512)], kv_ps[t])

        # stream pools open after the projection pool closes so they can
        # reuse its SBUF space; their DMAs are queued behind it anyway
        sbK = ctx.enter_context(tc.tile_pool(name="sbK", bufs=7))
        sbV = ctx.enter_context(tc.tile_pool(name="sbV", bufs=6))
        sbB = ctx.enter_context(tc.tile_pool(name="sbB", bufs=3))
        sb2 = ctx.enter_context(tc.tile_pool(name="sb2", bufs=2))
        sb3 = ctx.enter_context(tc.tile_pool(name="sb3", bufs=3))
        sc = ctx.enter_context(tc.tile_pool(name="sc", bufs=4))
        scm = ctx.enter_context(tc.tile_pool(name="scm", bufs=4))
        nsb = ctx.enter_context(tc.tile_pool(name="nsb", bufs=2))

        # ================= build + attention, interleaved ==============
        with ExitStack() as bctx:
            sim_pool = bctx.enter_context(tc.tile_pool(name="simp", bufs=2, space="PSUM"))
            acc_pool = bctx.enter_context(tc.tile_pool(name="accp", bufs=2, space="PSUM"))
            mem_ps = bctx.enter_context(tc.tile_pool(name="memps", bufs=1, space="PSUM"))
            pso = bctx.enter_context(tc.tile_pool(name="pso", bufs=1, space="PSUM"))

            # norm rows for ALL chunks upfront: keeps the Sqrt activations
            # contiguous (one act-table load) before the Exp stream starts
            rsq_c = [persist.tile([34, 512], F32R, name=f"rsq{i}")
                     for i in range(nq)]
            for c2 in range(nq):
                nc.scalar.dma_start(out=rsq_c[c2], in_=rsqz_d)
            for c2 in range(nq):
                sl = bass.ts(c2, 512)
                sqq = nsb.tile([P, 512], F32R, tag="sq", name=f"sqq{c2}")
                nc.vector.tensor_tensor(out=sqq, in0=qt_sb[:, sl],
                                        in1=qt_sb[:, sl], op=OP.mult)
                sqk = nsb.tile([P, 512], F32, tag="sqk", name=f"sqk{c2}")
                nc.vector.tensor_tensor(out=sqk, in0=kvt_sb[:, sl],
                                        in1=kvt_sb[:, sl], op=OP.mult)
                nrm_ps = mem_ps.tile([34, 512], F32, tag="at4", name=f"nrm{c2}")
                nc.tensor.matmul(nrm_ps[0:2, :], lhsT=Lq, rhs=sqq,
                                 start=True, stop=True)
                nc.tensor.matmul(nrm_ps[32:33, :], lhsT=onesk, rhs=sqk,
                                 start=True, stop=True)
                rcp = nsb.tile([33, 512], F32, tag="rcp", name=f"rcp{c2}")
                nc.vector.reciprocal(rcp[0:2, :], nrm_ps[0:2, :])
                nc.vector.reciprocal(rcp[32:33, :], nrm_ps[32:33, :])
                nc.scalar.activation(out=rsq_c[c2][0:2, :], in_=rcp[0:2, :],
                                     func=ACTF.Sqrt)
                nc.scalar.activation(out=rsq_c[c2][32:33, :], in_=rcp[32:33, :],
                                     func=ACTF.Sqrt)

            def build(c2):
                """qhT / kh2T / vb for chunk c2 (tiles 4c2..4c2+4)."""
                sl = bass.ts(c2, 512)
                rsq = rsq_c[c2]
                rqb_ps = mem_ps.tile([P, 512], F32, tag="E", name=f"rqb{c2}")
                nc.tensor.matmul(rqb_ps, lhsT=Lsel, rhs=rsq, start=True, stop=True)
                with nc.allow_low_precision(reason="qhat f16"):
                    nc.vector.tensor_tensor(out=qhT_c[c2], in0=qt_sb[:, sl],
                                            in1=rqb_ps, op=OP.mult)
                rkb_ps = pso.tile([DH, 512], F32, tag="ops", name=f"rkb{c2}")
                nc.tensor.matmul(rkb_ps, lhsT=Lk, rhs=rsq, start=True, stop=True)
                khT = nsb.tile([DH, 512], F16, tag="khT", name=f"khT{c2}")
                with nc.allow_low_precision(reason="khat f16"):
                    nc.vector.tensor_tensor(out=khT, in0=kvt_sb[0:DH, sl],
                                            in1=rkb_ps, op=OP.mult)
                kd_ps = mem_ps.tile([P, 512], F32, tag="moT", name=f"kd{c2}")
                nc.tensor.matmul(kd_ps, lhsT=Ldup, rhs=khT, start=True, stop=True)
                nc.vector.tensor_copy(kh2T_c[c2], kd_ps)
                for t in range(4 * c2, 4 * c2 + 4):
                    vbT_ps = sim_pool.tile([P, DH], F32, tag="sim",
                                           name=f"vbT{t}")
                    nc.tensor.transpose(vbT_ps, kvt_sb[DH:P, bass.ts(t, P)],
                                        id_f[DH:P, DH:P])
                    nc.vector.tensor_copy(vb_t[t][:, 0:DH], vbT_ps)
                    nc.vector.memset(vb_t[t][:, DH:DH + 1], 1.0)

            def prefetch(c):
                nkt = 4 * c + 4
                bias_ts = []
                for h in range(HPC):
                    bt = sbB.tile([P, 16, 512], FP8, tag="biasT",
                                  name=f"bt{c}{h}")
                    nc.gpsimd.dma_start(
                        out=bt[:, 0:nkt, :],
                        in_=bias_d[h, c, 0:nkt * P, :].rearrange(
                            "(t p) q -> p t q", p=P))
                    bias_ts.append(bt)
                memk_t = []
                memv_t = []
                for g in range(4):
                    it = 4 * c + g
                    mk = sbK.tile([P, KRET * P], F16, tag="memk", name=f"mk{it}")
                    (nc.sync if g % 2 == 0 else nc.gpsimd).dma_start(
                        out=mk, in_=memkT_d[it])
                    memk_t.append(mk)
                for g in range(4):
                    it = 4 * c + g
                    mv = sbV.tile([P, HPC, KRET, DH + 1], BF16, tag="memv",
                                  name=f"mv{it}")
                    (nc.gpsimd if g % 2 == 0 else nc.sync).dma_start(
                        out=mv, in_=memvp_d[it])
                    memv_t.append(mv)
                return bias_ts, memk_t, memv_t

            def attn_head(c, h, bias_t, memk_t, memv_t):
                hs = bass.ts(h, DH)
                nkt = 4 * c + 4
                acc_t = acc_pool.tile([P, 4, DH + 1], F32, tag="acc",
                                      name=f"acc{c}{h}")
                acc = [acc_t[:, g, :] for g in range(4)]

                E_ps = mem_ps.tile([P, 4, ng, 4], F32, tag="E", name=f"E{c}{h}")
                E_sb = scm.tile([P, 4, ng, 4], BF16, tag="E_sb")
                prev_e = [None]

                def local_pv(kt, e_sb):
                    for g in range(max(0, kt - 4 * c), 4):
                        nc.tensor.matmul(acc[g], lhsT=e_sb[:, bass.ts(g, P)],
                                         rhs=vb_t[kt],
                                         start=(kt == 0), stop=False)

                def mem_sims():
                    for g in range(4):
                        for g2 in range(ng):
                            nc.tensor.matmul(
                                E_ps[:, g, g2, :],
                                lhsT=memk_t[g][hs, bass.ts(g2, P)],
                                rhs=qhT_c[c][hs, g * P + 4 * g2:
                                             g * P + 4 * g2 + 4],
                                start=True, stop=True)

                for kt in range(nkt):
                    # diagonal key-tiles are invisible to queries before
                    # column q0 = (kt-4c)*128: trim sims/bias/exp to the
                    # visible suffix
                    q0 = max(0, kt - 4 * c) * P
                    sim_ps = sim_pool.tile([P, 512], F32, tag="sim")
                    nc.tensor.matmul(sim_ps[:, q0:512],
                                     lhsT=kh2T_c[kt // 4][hs, bass.ts(kt % 4, P)],
                                     rhs=qhT_c[c][hs, q0:512],
                                     start=True, stop=False)
                    nc.tensor.matmul(sim_ps[:, q0:512], lhsT=id_8,
                                     rhs=bias_t[:, kt, q0:512],
                                     start=False, stop=True)
                    e_sb = sb3.tile([P, 512], BF16, tag="e")
                    nc.scalar.activation(out=e_sb[:, q0:512],
                                         in_=sim_ps[:, q0:512], func=ACTF.Exp,
                                         bias=negm_sb)
                    if prev_e[0] is not None:
                        local_pv(*prev_e[0])
                    prev_e[0] = (kt, e_sb)
                    if kt == 1:
                        mem_sims()
                    if kt == 2:
                        # mem exp + cross-query mask, early so the PE's
                        # mem-PV matmuls below never stall on Act
                        with nc.allow_low_precision(reason="mem wts bf16"):
                            nc.scalar.activation(out=E_sb, in_=E_ps,
                                                 func=ACTF.Exp, bias=negm_sb)
                            nc.vector.tensor_tensor(
                                out=E_sb, in0=E_sb,
                                in1=bcast_mid(maskc_sb, 4 * ng), op=OP.mult)
                local_pv(*prev_e[0])

                # ---- mem PV + tails, stage-major -------------------------
                moT_ps = mem_ps.tile([DH + 1, 4, P], F32, tag="moT",
                                     name=f"moT{c}{h}")
                for g in range(4):
                    for g2 in range(ng):
                        nc.tensor.matmul(moT_ps[:, g, bass.ts(g2, 4)],
                                         lhsT=memv_t[g][:, h, g2, :],
                                         rhs=E_sb[:, g, g2, :],
                                         start=True, stop=True)
                moT_sb = scm.tile([DH + 1, 4, P], F32, tag="moT_sb")
                nc.vector.tensor_copy(moT_sb, moT_ps)
                for g in range(4):
                    # transpose-accumulate mem output onto the local PV
                    # accumulator: acc[g] += moT_sb[:, g, :]^T
                    nc.tensor.matmul(acc[g], lhsT=moT_sb[:, g, :],
                                     rhs=id_f[0:DH + 1, 0:DH + 1],
                                     is_transpose=True,
                                     start=False, stop=True)
                rzs = sc.tile([P, 4], F32, tag="rzs")
                for g in range(4):
                    nc.vector.reciprocal(rzs[:, g:g + 1], acc[g][:, DH:DH + 1])
                for g in range(4):
                    nc.vector.tensor_scalar_mul(a_t[4 * c + g][:, hs],
                                                acc[g][:, 0:DH],
                                                rzs[:, g:g + 1])
                if h == HPC - 1:
                    # ---- output projection, stage-major -----------------
                    at4_ps = mem_ps.tile([P, 4 * P], F16, tag="at4",
                                         name=f"at4{c}")
                    for g in range(4):
                        nc.tensor.transpose(at4_ps[:, bass.ts(g, P)],
                                            a_t[4 * c + g], id_h)
                    at4_sb = sb2.tile([P, 4 * P], F16, tag="at4_sb")
                    nc.vector.tensor_copy(at4_sb, at4_ps)
                    for g in range(4):
                        it = 4 * c + g
                        o_ps = pso.tile([P, DIM], F32, tag="ops",
                                        name=f"o{it}")
                        nc.tensor.matmul(o_ps, lhsT=at4_sb[:, bass.ts(g, P)],
                                         rhs=wout_sb, start=True, stop=True)
                        o_sb = sb2.tile([P, DIM], F16, tag="osb")
                        with nc.allow_low_precision(reason="out f16"):
                            if c == nq - 1 and g % 2 == 1:
                                nc.scalar.copy(o_sb, o_ps)
                            else:
                                nc.vector.tensor_copy(o_sb, o_ps)
                        nc.sync.dma_start(out=out_d[bass.ts(it, P), :],
                                          in_=o_sb)

            pf = [prefetch(0), prefetch(1)]
            build(0)
            build(1)
            for c in range(nq):
                bias_ts, memk_t, memv_t = pf[0]
                attn_head(c, 0, bias_ts[0], memk_t, memv_t)
                if c + 2 < nq:
                    pf.append(prefetch(c + 2))
                    build(c + 2)
                pf.pop(0)
                attn_head(c, 1, bias_ts[1], memk_t, memv_t)

    nc.compile()
    return nc


# ===================== host side =====================================

def prep_core_inputs(x, mem_kv, mem_mask, rel_pos_bias, Wq, Wkv, Wout,
                     scale_param):
    """Shard the full inputs into 8 per-core input maps."""
    b, n, dim = x.shape
    h = scale_param.shape[0]
    nq = n // 512
    nt = n // P
    bf = ml_dtypes.bfloat16

    scales = np.exp(np.asarray(scale_param, np.float32).reshape(h))
    xt = [np.ascontiguousarray(np.asarray(x[i], np.float32).T).astype(np.float16)
          for i in range(b)]
    biasr = np.asarray(rel_pos_bias[0], np.float32).copy()
    iu = np.triu_indices(n, 1)
    biasr[:, iu[0], iu[1]] = -240.0
    # transposed/blocked: biasT[h, c, j, i'] = bias[h, 512c+i', j]
    biasT = np.ascontiguousarray(
        biasr.reshape(h, nq, 512, n).transpose(0, 1, 3, 2)).astype(
            ml_dtypes.float8_e4m3)

    memk = np.asarray(mem_kv[..., 0, :], np.float32)   # b h n k d
    memv = np.asarray(mem_kv[..., 1, :], np.float32)   # b h n k d
    mask = np.asarray(mem_mask)                        # b h n k

    # memkT[b, h, t, d, (g,j,k)] = memk[b, h, 128t+4g+j, k, d]
    mk5 = memk.reshape(b, h, nt, KRET, 4, KRET, DH)    # b h t g j k d
    memkT = np.ascontiguousarray(
        mk5.transpose(0, 1, 2, 6, 3, 4, 5)             # b h t d g j k
    ).reshape(b, h, nt, DH, KRET * P).astype(np.float16)

    # memvp[b, t, (j,k), h, g, d] = memv[b, h, 128t+4g+j, k, d] * mask
    mv5 = memv.reshape(b, h, nt, KRET, 4, KRET, DH)    # b h t g j k d
    mvp = np.empty((b, nt, 4, KRET, h, KRET, DH + 1), np.float32)  # b t j k h g d
    mvp[..., 0:DH] = mv5.transpose(0, 2, 4, 5, 1, 3, 6)
    mvp[..., DH] = 1.0
    if not mask.all():
        m5 = mask.reshape(b, h, nt, KRET, 4, KRET)     # b h t g j k
        mvp *= m5.transpose(0, 2, 4, 5, 1, 3)[..., None]
    memvp = np.ascontiguousarray(mvp.reshape(b, nt, P, h, KRET, DH + 1)).astype(bf)

    maskc = np.zeros((P, 4), bf)
    for j in range(4):
        maskc[j * KRET:(j + 1) * KRET, j] = 1.0

    onesk = np.zeros((P, 1), np.float32)
    onesk[0:DH, 0] = 1.0
    Lsel = np.zeros((34, P), np.float32)
    Lsel[0, 0:DH] = 1.0
    Lsel[1, DH:P] = 1.0
    Lk = np.zeros((34, DH), np.float32)
    Lk[32, :] = 1.0
    Ldup = np.zeros((DH, P), np.float16)
    for d in range(DH):
        Ldup[d, d] = 1.0
        Ldup[d, DH + d] = 1.0

    Wq16 = np.asarray(Wq, np.float32).astype(np.float16)
    Wkv16 = np.asarray(Wkv, np.float32).astype(np.float16)
    Wout16 = np.asarray(Wout, np.float32).astype(np.float16)

    in_maps = []
    for c in range(NCORES):
        bi, hg = divmod(c, NCORES // b)
        hs = slice(HPC * hg, HPC * hg + HPC)
        # memkT per core: [nt, (h,d), 4096]
        mkc = np.ascontiguousarray(
            memkT[bi, hs].transpose(1, 0, 2, 3).reshape(nt, P, KRET * P))
        Lqc = np.zeros((P, HPC), np.float32)
        for hh in range(HPC):
            Lqc[DH * hh:DH * (hh + 1), hh] = 1.0 / scales[HPC * hg + hh] ** 2
        in_maps.append({
            "xt": xt[bi],
            "wq": np.ascontiguousarray(Wq16[:, HPC * DH * hg: HPC * DH * (hg + 1)]),
            "wkv": Wkv16,
            "wout": np.ascontiguousarray(Wout16[HPC * DH * hg: HPC * DH * (hg + 1), :]),
            "Lq": Lqc,
            "onesk": onesk,
            "Lsel": Lsel,
            "Lk": Lk,
            "Ldup": Ldup,
            "rsqz": np.zeros((34, 512), np.float32),
            "memkT": mkc,
            "memvp": np.ascontiguousarray(memvp[bi, :, :, hs]),
            "maskc": maskc,
            "biasT": np.ascontiguousarray(biasT[hs]),
        })
    return in_maps


_NC_CACHE = {}


def kernel(x, mem_kv, mem_mask, rel_pos_bias, Wq, Wkv, Wout, scale_param,
           trace=False):
    from concourse.bass_utils import run_bass_kernel_spmd

    b, n, dim = x.shape
    in_maps = prep_core_inputs(x, mem_kv, mem_mask, rel_pos_bias, Wq, Wkv,
                               Wout, scale_param)
    if n not in _NC_CACHE:
        _NC_CACHE[n] = build_nc(n)
    nc = _NC_CACHE[n]
    res = run_bass_kernel_spmd(nc, in_maps, core_ids=list(range(NCORES)),
                               trace=trace)
    outs = [r["out"] for r in res.results]
    full = np.zeros((b, n, dim), np.float32)
    g = NCORES // b
    for c in range(NCORES):
        full[c // g] += outs[c].astype(np.float32)
    if trace:
        kernel.last_results = res
    return full
